# revision 5
# baseline (speedup 1.0000x reference)
"""Trainium2 Bass kernel for nn_CATAggregator (linear attention over shared
prototypes + LN + FFN), data-parallel over N = B*H*W on 8 NeuronCores.

Self-contained: hardcodes shapes from the problem spec.

Layout: feature-major per core — activations live as [C=128 partitions,
tokens free], token = (n_local, t) with t fastest. Each core gets one
quarter-batch half-height slab: core i -> b = i//2, h in [12*(i%2), +12).

Per 512-token tile: Qproj (+guidance-add via replicated-identity matmul),
elu+1 via exp(min(q,0))+relu(q), block-diagonal linear attention (KV and
ksum folded into 128x128 / 128x4 stationary matrices on host), LN stats via
ones-vector matmuls batched 4 tiles/PSUM bank at 32-partition offsets,
rstd/1-over-z via ACT ln+exp (single activation table set), per-token-scalar
broadcasts via rank-1 matmuls, FFN in 4 128-chunks with gelu (b1 folded into
the ACT bias), residuals folded into the FFN2 PSUM accumulation (identity
matmul + rank-1 mean-correction).
"""
import os
import numpy as np

B, T, C, Hs, Ws = 4, 128, 128, 24, 24
G, P, NH = 128, 32, 4
HD = C // NH
EPS_ATTN, EPS_LN = 1e-6, 1e-5
NCORES = 8
F = 512                      # tokens per tile (= one fp32 PSUM bank)
NT_CORE = (B * Hs * Ws // NCORES) * T   # 288 * 128 = 36864 tokens per core
NTILES = NT_CORE // F        # 72
GROUP = 4                    # tiles per stats batch (4 x 32-partition slots)

_COMPILED = {}


def _np(v):
    return np.asarray(v, dtype=np.float32)


def build_consts(inputs):
    """Host-side precompute of all stationary matrices (fp64 for accuracy)."""
    Wq = np.asarray(inputs["Wq"], np.float64)
    bq = np.asarray(inputs["bq"], np.float64)
    Wk = np.asarray(inputs["Wk"], np.float64)
    bk = np.asarray(inputs["bk"], np.float64)
    Wv = np.asarray(inputs["Wv"], np.float64)
    bv = np.asarray(inputs["bv"], np.float64)
    protos = np.asarray(inputs["protos"], np.float64)[0]
    W1 = np.asarray(inputs["W1"], np.float64)
    b1 = np.asarray(inputs["b1"], np.float64)
    W2 = np.asarray(inputs["W2"], np.float64)
    g1 = np.asarray(inputs["ln1_g"], np.float64)

    k = protos @ Wk.T + bk
    v = protos @ Wv.T + bv
    kf = np.where(k > 0, k, np.expm1(k)) + 1.0          # elu(k)+1
    kf = kf.reshape(P, NH, HD)
    vr = v.reshape(P, NH, HD)
    KV = np.einsum('phd,phv->hdv', kf, vr)              # /P and *P cancel
    ksum = kf.sum(axis=0)                                # (NH, HD)

    KVblk = np.zeros((C, C), np.float32)
    KSblk = np.zeros((C, NH), np.float32)
    SelRep = np.zeros((C, C), np.float32)                # rows repl. at 32-bnds
    for h in range(NH):
        sl = slice(h * HD, (h + 1) * HD)
        KVblk[sl, sl] = KV[h]
        KSblk[sl, h] = ksum[h]
        for j in range(4):
            SelRep[32 * j + h, sl] = 1.0

    Irep = np.tile(np.eye(T, dtype=np.float32), (1, F // T))   # (128, 512)
    statsONE = np.full((C, 1), 1.0 / C, np.float32)
    statsFULL = np.zeros((C, C), np.float32)
    statsFULL[:, 0] = 1.0 / C
    KSfull = np.zeros((C, C), np.float32)
    KSfull[:, :NH] = KSblk
    ONESrows = np.ones((C, C), np.float32)                      # rank-1 lhsT rows
    NEGG1rows = np.tile(-g1[None, :].astype(np.float32), (C, 1))

    W1T = np.concatenate([W1[c * 128:(c + 1) * 128, :].T
                          for c in range(4)], axis=1).astype(np.float32)  # (128,512)
    B1c = np.stack([b1[c * 128:(c + 1) * 128] for c in range(4)],
                   axis=1).astype(np.float32)                             # (128,4)
    W2T = np.concatenate([W2[:, c * 128:(c + 1) * 128].T
                          for c in range(4)], axis=1).astype(np.float32)  # (128,512)
    return dict(
        WqxT=Wq[:, :C].T.astype(np.float32).copy(),
        Wqg=Wq[:, C:].astype(np.float32).copy(),
        bq=bq.astype(np.float32),
        KVblk=KVblk, KSblk=KSblk, SelRep=SelRep, Irep=Irep,
        statsONE=statsONE, statsFULL=statsFULL, KSfull=KSfull,
        ONESrows=ONESrows, NEGG1rows=NEGG1rows,
        W1T=W1T, B1c=B1c, W2T=W2T,
        I128=np.eye(C, dtype=np.float32),
    )


def build_bass(ntiles=NTILES):
    """Build the SPMD Bacc program for one core over ntiles*F tokens."""
    import concourse.bacc as bacc
    import concourse.mybir as mybir
    import concourse.tile as tile

    fp32 = mybir.dt.float32
    ntok = ntiles * F
    nc = bacc.Bacc("TRN2", target_bir_lowering=False, debug=False,
                   num_devices=NCORES)

    xT = nc.dram_tensor("xT", [C, ntok], fp32, kind="ExternalInput")
    outT = nc.dram_tensor("outT", [C, ntok], fp32, kind="ExternalOutput")
    d_consts = {}
    for name, shape in [
            ("WqxT", [C, C]), ("qgT", [T, C]), ("KVblk", [C, C]),
            ("KSblk", [C, NH]), ("SelRep", [C, C]), ("Irep", [T, F]),
            ("statsONE", [C, 1]), ("statsFULL", [C, C]), ("KSfull", [C, C]),
            ("ONESrows", [C, C]), ("NEGG1rows", [C, C]),
            ("W1T", [C, 4 * C]), ("B1c", [C, 4]), ("W2T", [C, 4 * C]),
            ("I128", [C, C])]:
        d_consts[name] = nc.dram_tensor(name, shape, fp32, kind="ExternalInput")

    Exp = mybir.ActivationFunctionType.Exp
    Ln = mybir.ActivationFunctionType.Ln
    Gelu = mybir.ActivationFunctionType.Gelu
    Copy = mybir.ActivationFunctionType.Copy
    MULT = mybir.AluOpType.mult
    SUB = mybir.AluOpType.subtract
    ADD = mybir.AluOpType.add

    with tile.TileContext(nc) as tc:
        import contextlib
        ctx = contextlib.ExitStack()
        with ctx:
            cpool = ctx.enter_context(tc.tile_pool(name="consts", bufs=1))
            xp = ctx.enter_context(tc.tile_pool(name="xp", bufs=6))
            sp = ctx.enter_context(tc.tile_pool(name="sp", bufs=2))
            wp = ctx.enter_context(tc.tile_pool(name="wp", bufs=6))
            ap_ = ctx.enter_context(tc.tile_pool(name="ap", bufs=6))
            hp = ctx.enter_context(tc.tile_pool(name="hp", bufs=2))
            op = ctx.enter_context(tc.tile_pool(name="op", bufs=2))
            rp = ctx.enter_context(tc.tile_pool(name="rp", bufs=2))
            ps = ctx.enter_context(tc.tile_pool(name="ps", bufs=1, space="PSUM"))
            psf = ctx.enter_context(tc.tile_pool(name="psf", bufs=2, space="PSUM"))

            # --- resident constants ---
            cb = {}
            for name, t in d_consts.items():
                shape = list(t.shape)
                ct = cpool.tile(shape, fp32, tag=f"c_{name}")
                nc.sync.dma_start(out=ct[:], in_=t[:, :])
                cb[name] = ct
            epsA = cpool.tile([C, 1], fp32, tag="epsA")
            nc.vector.memset(epsA[:], EPS_ATTN)
            epsL = cpool.tile([C, 1], fp32, tag="epsL")
            nc.vector.memset(epsL[:], EPS_LN)

            ACT_EXPSET = []   # ACT insts in the ln/exp table set, in order
            ACT_GELUS = []    # gelu ACT insts

            ngroups = ntiles // GROUP
            for g in range(ngroups):
                # group-level PSUM stats banks (4 tiles @ 32-bnd slots)
                bank_mu1 = ps.tile([C, F], fp32, tag="mu")
                bank_msq1 = ps.tile([C, F], fp32, tag="msq")
                bank_z = ps.tile([C, F], fp32, tag="z")

                tiles = range(g * GROUP, (g + 1) * GROUP)
                x_t, x2_t, qf_t = {}, {}, {}
                for t in tiles:
                    j = t % GROUP
                    sl = slice(t * F, (t + 1) * F)
                    xt = xp.tile([C, F], fp32, tag="x")
                    nc.sync.dma_start(out=xt[:], in_=xT[:, sl])
                    x_t[t] = xt
                    # Qproj + guidance
                    psQ = ps.tile([C, F], fp32, tag="qa")
                    nc.tensor.matmul(psQ[:], cb["WqxT"][:], xt[:],
                                     start=True, stop=False)
                    nc.tensor.matmul(psQ[:], cb["qgT"][:], cb["Irep"][:],
                                     start=False, stop=True)
                    # stats1
                    if j == 0:
                        nc.tensor.matmul(bank_mu1[:, :], cb["statsFULL"][:], xt[:])
                    else:
                        nc.tensor.matmul(bank_mu1[32 * j:32 * j + 1, :],
                                         cb["statsONE"][:], xt[:],
                                         tile_position=(0, 32 * j))
                    x2 = sp.tile([C, F], fp32, tag="x2")
                    nc.gpsimd.tensor_tensor(x2[:], xt[:], xt[:], MULT)
                    x2_t[t] = x2
                    if j == 0:
                        nc.tensor.matmul(bank_msq1[:, :], cb["statsFULL"][:], x2[:])
                    else:
                        nc.tensor.matmul(bank_msq1[32 * j:32 * j + 1, :],
                                         cb["statsONE"][:], x2[:],
                                         tile_position=(0, 32 * j))
                    # elu+1 = exp(min(q,0)) + relu(q)
                    m = sp.tile([C, F], fp32, tag="m")
                    nc.vector.tensor_scalar_min(m[:], psQ[:], 0.0)
                    r = sp.tile([C, F], fp32, tag="r")
                    nc.vector.tensor_scalar_max(r[:], psQ[:], 0.0)
                    e = sp.tile([C, F], fp32, tag="e")
                    ACT_EXPSET.append(nc.scalar.activation(e[:], m[:], Exp))
                    qf = sp.tile([C, F], fp32, tag="qf")
                    nc.gpsimd.tensor_tensor(qf[:], e[:], r[:], ADD)
                    qf_t[t] = qf
                    # attention numerator + z
                    psA = ps.tile([C, F], fp32, tag="qa")
                    nc.tensor.matmul(psA[:], cb["KVblk"][:], qf[:])
                    if j == 0:
                        nc.tensor.matmul(bank_z[:, :], cb["KSfull"][:], qf[:])
                    else:
                        nc.tensor.matmul(bank_z[32 * j:32 * j + NH, :],
                                         cb["KSblk"][:], qf[:],
                                         tile_position=(0, 32 * j))
                    attnS = ap_.tile([C, F], fp32, tag="attnS")
                    nc.vector.tensor_copy(attnS[:], psA[:])
                    x_t[t + 1000] = attnS  # stash (dict reuse)

                # ---- rowmath phase 1 (zinv, rstd1, u1) ----
                lnz = rp.tile([C, F], fp32, tag="lnz")
                ACT_EXPSET.append(nc.scalar.activation(
                    lnz[:], bank_z[:], Ln, bias=epsA[:]))
                zinvR = rp.tile([C, F], fp32, tag="zinvR")
                ACT_EXPSET.append(nc.scalar.activation(
                    zinvR[:], lnz[:], Exp, scale=-1.0))
                muS = rp.tile([C, F], fp32, tag="muS")
                ACT_EXPSET.append(nc.scalar.activation(muS[:], bank_mu1[:], Copy))
                musq = rp.tile([C, F], fp32, tag="musq")
                nc.vector.tensor_tensor(musq[:], muS[:], muS[:], MULT)
                var1 = rp.tile([C, F], fp32, tag="var1")
                nc.vector.tensor_tensor(var1[:], bank_msq1[:], musq[:], SUB)
                lnv1 = rp.tile([C, F], fp32, tag="lnv1")
                ACT_EXPSET.append(nc.scalar.activation(
                    lnv1[:], var1[:], Ln, bias=epsL[:]))
                rstd1R = rp.tile([C, F], fp32, tag="rstd1R")
                ACT_EXPSET.append(nc.scalar.activation(
                    rstd1R[:], lnv1[:], Exp, scale=-0.5))
                u1R = rp.tile([C, F], fp32, tag="u1R")
                nc.vector.tensor_tensor(u1R[:], muS[:], rstd1R[:], MULT)

                # ---- per-tile: zb/A1 bcasts, w, stats2 ----
                bank_mu2 = ps.tile([C, F], fp32, tag="mu")
                bank_msq2 = ps.tile([C, F], fp32, tag="msq")
                w_t, w2_t = {}, {}
                for t in tiles:
                    j = t % GROUP
                    attnS = x_t[t + 1000]
                    psZB = ps.tile([C, F], fp32, tag="bc")
                    nc.tensor.matmul(psZB[:], cb["SelRep"][32 * j:32 * j + NH, :],
                                     zinvR[32 * j:32 * j + NH, :],
                                     tile_position=(32 * j, 0))
                    attn = sp.tile([C, F], fp32, tag="attn")
                    nc.vector.tensor_tensor(attn[:], attnS[:], psZB[:], MULT)
                    psA1 = ps.tile([C, F], fp32, tag="bc")
                    nc.tensor.matmul(psA1[:], cb["ONESrows"][32 * j:32 * j + 1, :],
                                     rstd1R[32 * j:32 * j + 1, :],
                                     tile_position=(32 * j, 0))
                    t1 = sp.tile([C, F], fp32, tag="t1")
                    nc.vector.tensor_tensor(t1[:], x_t[t][:], psA1[:], MULT)
                    w = wp.tile([C, F], fp32, tag="w")
                    nc.vector.tensor_tensor(w[:], attn[:], t1[:], ADD)
                    w_t[t] = w
                    if j == 0:
                        nc.tensor.matmul(bank_mu2[:, :], cb["statsFULL"][:], w[:])
                    else:
                        nc.tensor.matmul(bank_mu2[32 * j:32 * j + 1, :],
                                         cb["statsONE"][:], w[:],
                                         tile_position=(0, 32 * j))
                    w2 = sp.tile([C, F], fp32, tag="w2")
                    nc.gpsimd.tensor_tensor(w2[:], w[:], w[:], MULT)
                    w2_t[t] = w2
                    if j == 0:
                        nc.tensor.matmul(bank_msq2[:, :], cb["statsFULL"][:], w2[:])
                    else:
                        nc.tensor.matmul(bank_msq2[32 * j:32 * j + 1, :],
                                         cb["statsONE"][:], w2[:],
                                         tile_position=(0, 32 * j))

                # ---- rowmath phase 2 (mu2w copy, rstd2) ----
                mu2wS = rp.tile([C, F], fp32, tag="mu2wS")
                ACT_EXPSET.append(nc.scalar.activation(mu2wS[:], bank_mu2[:], Copy))
                musq2 = rp.tile([C, F], fp32, tag="musq")
                nc.vector.tensor_tensor(musq2[:], mu2wS[:], mu2wS[:], MULT)
                var2 = rp.tile([C, F], fp32, tag="var1")
                nc.vector.tensor_tensor(var2[:], bank_msq2[:], musq2[:], SUB)
                lnv2 = rp.tile([C, F], fp32, tag="lnv1")
                ACT_EXPSET.append(nc.scalar.activation(
                    lnv2[:], var2[:], Ln, bias=epsL[:]))
                rstd2R = rp.tile([C, F], fp32, tag="rstd2R")
                ACT_EXPSET.append(nc.scalar.activation(
                    rstd2R[:], lnv2[:], Exp, scale=-0.5))

                # ---- per-tile: LN2 apply, FFN, residuals, store ----
                for t in tiles:
                    j = t % GROUP
                    w = w_t[t]
                    psM2 = ps.tile([C, F], fp32, tag="bc")
                    nc.tensor.matmul(psM2[:], cb["ONESrows"][32 * j:32 * j + 1, :],
                                     mu2wS[32 * j:32 * j + 1, :],
                                     tile_position=(32 * j, 0))
                    ln2p = sp.tile([C, F], fp32, tag="ln2p")
                    nc.vector.tensor_tensor(ln2p[:], w[:], psM2[:], SUB)
                    psA2 = ps.tile([C, F], fp32, tag="bc")
                    nc.tensor.matmul(psA2[:], cb["ONESrows"][32 * j:32 * j + 1, :],
                                     rstd2R[32 * j:32 * j + 1, :],
                                     tile_position=(32 * j, 0))
                    ln2 = sp.tile([C, F], fp32, tag="ln2")
                    nc.vector.tensor_tensor(ln2[:], ln2p[:], psA2[:], MULT)

                    psOut = ps.tile([C, F], fp32, tag="out")
                    for c in range(4):
                        psF1 = psf.tile([C, F], fp32, tag="f1")
                        nc.tensor.matmul(psF1[:], cb["W1T"][:, 128 * c:128 * (c + 1)],
                                         ln2[:])
                        h = hp.tile([C, F], fp32, tag="h")
                        ACT_GELUS.append(nc.scalar.activation(
                            h[:], psF1[:], Gelu, bias=cb["B1c"][:, c:c + 1]))
                        nc.tensor.matmul(psOut[:], cb["W2T"][:, 128 * c:128 * (c + 1)],
                                         h[:], start=(c == 0), stop=False,
                                         skip_group_check=True)
                    nc.tensor.matmul(psOut[:], cb["I128"][:], w[:],
                                     start=False, stop=False, skip_group_check=True)
                    nc.tensor.matmul(psOut[:], cb["NEGG1rows"][32 * j:32 * j + 1, :],
                                     u1R[32 * j:32 * j + 1, :],
                                     tile_position=(32 * j, 0),
                                     start=False, stop=True, skip_group_check=True)
                    outS = op.tile([C, F], fp32, tag="outS")
                    nc.vector.tensor_copy(outS[:], psOut[:])
                    nc.sync.dma_start(out=outT[:, t * F:(t + 1) * F], in_=outS[:])

    nc.compile()
    return nc


def _shard_inputs(inputs, consts, ntiles=NTILES):
    """Build per-core in_maps (list of dicts)."""
    x = np.asarray(inputs["x"], np.float32)
    guidance = np.asarray(inputs["guidance"], np.float32)
    ntok = ntiles * F
    in_maps = []
    const_arrs = {k: consts[k] for k in
                  ("WqxT", "KVblk", "KSblk", "SelRep", "Irep", "statsONE",
                   "statsFULL", "KSfull", "ONESrows", "NEGG1rows", "W1T", "B1c",
                   "W2T", "I128")}
    for core in range(NCORES):
        b = core // 2
        h0 = 12 * (core % 2)
        xs = x[b, :, :, h0:h0 + 12, :]                 # (T,C,12,24)
        xc = np.ascontiguousarray(
            xs.transpose(1, 2, 3, 0).reshape(C, NT_CORE))[:, :ntok]
        qg = (guidance[b].astype(np.float64) @ consts["Wqg"].astype(np.float64).T
              + consts["bq"].astype(np.float64)).astype(np.float32)   # (T,C)
        m = {"xT": np.ascontiguousarray(xc), "qgT": qg}
        m.update(const_arrs)
        in_maps.append(m)
    return in_maps


def _unshard(results):
    out = np.empty((B, T, C, Hs, Ws), np.float32)
    for core in range(NCORES):
        b = core // 2
        h0 = 12 * (core % 2)
        o = results[core]["outT"]                       # (C, NT_CORE)
        o4 = o.reshape(C, 12, 24, T).transpose(3, 0, 1, 2)
        out[b, :, :, h0:h0 + 12, :] = o4
    return out


def _numpy_fallback(inputs):
    """Plain-numpy reference path (used only for nontrivial ln g/b)."""
    from scipy.special import erf
    x = np.asarray(inputs["x"], np.float64)
    guidance = np.asarray(inputs["guidance"], np.float64)
    i64 = {k: np.asarray(v, np.float64) for k, v in inputs.items()}
    b_, t_, c_, h_, w_ = x.shape
    n = b_ * h_ * w_
    xb = x.transpose(0, 3, 4, 1, 2).reshape(n, t_, c_)
    g = np.broadcast_to(guidance[:, None, None, :, :],
                        (b_, h_, w_, t_, guidance.shape[-1])).reshape(n, t_, -1)
    q = np.concatenate([xb, g], -1) @ i64["Wq"].T + i64["bq"]
    proto = i64["protos"][0]
    k = proto @ i64["Wk"].T + i64["bk"]
    v = proto @ i64["Wv"].T + i64["bv"]
    elu1 = lambda z: np.where(z > 0, z, np.expm1(z)) + 1.0
    qf = elu1(q.reshape(n, t_, NH, HD))
    kf = elu1(k.reshape(P, NH, HD))
    vv = v.reshape(P, NH, HD) / P
    KV = np.einsum('phd,phv->hdv', kf, vv)
    ksum = kf.sum(0)
    Z = 1.0 / (np.einsum('nlhd,hd->nlh', qf, ksum) + EPS_ATTN)
    out = np.einsum('nlhd,hdv->nlhv', qf, KV) * Z[..., None] * P
    out = out.reshape(n, t_, c_)
    ln = lambda z, gg, bb: ((z - z.mean(-1, keepdims=True))
                            / np.sqrt(z.var(-1, keepdims=True) + EPS_LN) * gg + bb)
    out = out + ln(xb, i64["ln1_g"], i64["ln1_b"])
    hdn = ln(out, i64["ln2_g"], i64["ln2_b"]) @ i64["W1"].T + i64["b1"]
    hdn = 0.5 * hdn * (1.0 + erf(hdn / np.sqrt(2.0)))
    out = out + hdn @ i64["W2"].T + i64["b2"]
    out = out.reshape(b_, h_, w_, t_, c_).transpose(0, 3, 4, 1, 2)
    return out.astype(np.float32)


def kernel(**inputs):
    g1 = np.asarray(inputs["ln1_g"]); b1 = np.asarray(inputs["ln1_b"])
    g2 = np.asarray(inputs["ln2_g"]); b2l = np.asarray(inputs["ln2_b"])
    if not (np.allclose(g1, 1) and np.allclose(g2, 1)
            and np.allclose(b1, 0) and np.allclose(b2l, 0)
            and np.allclose(np.asarray(inputs["b2"]), 0)):
        return _numpy_fallback(inputs)

    from concourse.bass_utils import run_bass_kernel_spmd
    consts = build_consts(inputs)
    key = NTILES
    if key not in _COMPILED:
        _COMPILED[key] = build_bass(NTILES)
    nc = _COMPILED[key]
    in_maps = _shard_inputs(inputs, consts)
    res = run_bass_kernel_spmd(nc, in_maps, list(range(NCORES)))
    return _unshard(res.results)


# revision 7
# speedup vs baseline: 1.0412x; 1.0412x over previous
"""Trainium2 Bass kernel for nn_CATAggregator (linear attention over shared
prototypes + LN + FFN), data-parallel over N = B*H*W on 8 NeuronCores.

Self-contained: hardcodes shapes from the problem spec.

Layout: feature-major per core — activations live as [C=128 partitions,
tokens free], token = (n_local, t) with t fastest. Each core gets one
quarter-batch half-height slab: core i -> b = i//2, h in [12*(i%2), +12).

Per 512-token tile: Qproj (+guidance-add via replicated-identity matmul),
elu+1 via exp(min(q,0))+relu(q), block-diagonal linear attention (KV and
ksum folded into 128x128 / 128x4 stationary matrices on host), LN stats via
ones-vector matmuls batched 4 tiles/PSUM bank at 32-partition offsets,
rstd/1-over-z via ACT ln+exp (single activation table set), per-token-scalar
broadcasts via rank-1 matmuls, FFN in 4 128-chunks with gelu (b1 folded into
the ACT bias), residuals folded into the FFN2 PSUM accumulation (identity
matmul + rank-1 mean-correction).
"""
import os
import numpy as np

B, T, C, Hs, Ws = 4, 128, 128, 24, 24
G, P, NH = 128, 32, 4
HD = C // NH
EPS_ATTN, EPS_LN = 1e-6, 1e-5
NCORES = 8
F = 512                      # tokens per tile (= one fp32 PSUM bank)
NT_CORE = (B * Hs * Ws // NCORES) * T   # 288 * 128 = 36864 tokens per core
NTILES = NT_CORE // F        # 72
GROUP = 4                    # tiles per stats batch (4 x 32-partition slots)

_COMPILED = {}


def _np(v):
    return np.asarray(v, dtype=np.float32)


def build_consts(inputs):
    """Host-side precompute of all stationary matrices (fp64 for accuracy)."""
    Wq = np.asarray(inputs["Wq"], np.float64)
    bq = np.asarray(inputs["bq"], np.float64)
    Wk = np.asarray(inputs["Wk"], np.float64)
    bk = np.asarray(inputs["bk"], np.float64)
    Wv = np.asarray(inputs["Wv"], np.float64)
    bv = np.asarray(inputs["bv"], np.float64)
    protos = np.asarray(inputs["protos"], np.float64)[0]
    W1 = np.asarray(inputs["W1"], np.float64)
    b1 = np.asarray(inputs["b1"], np.float64)
    W2 = np.asarray(inputs["W2"], np.float64)
    g1 = np.asarray(inputs["ln1_g"], np.float64)

    k = protos @ Wk.T + bk
    v = protos @ Wv.T + bv
    kf = np.where(k > 0, k, np.expm1(k)) + 1.0          # elu(k)+1
    kf = kf.reshape(P, NH, HD)
    vr = v.reshape(P, NH, HD)
    KV = np.einsum('phd,phv->hdv', kf, vr)              # /P and *P cancel
    ksum = kf.sum(axis=0)                                # (NH, HD)

    KVblk = np.zeros((C, C), np.float32)
    KSblk = np.zeros((C, NH), np.float32)
    SelRep = np.zeros((C, C), np.float32)                # rows repl. at 32-bnds
    for h in range(NH):
        sl = slice(h * HD, (h + 1) * HD)
        KVblk[sl, sl] = KV[h]
        KSblk[sl, h] = ksum[h]
        for j in range(4):
            SelRep[32 * j + h, sl] = 1.0

    Irep = np.tile(np.eye(T, dtype=np.float32), (1, F // T))   # (128, 512)
    statsONE = np.full((C, 1), 1.0 / C, np.float32)
    statsFULL = np.zeros((C, C), np.float32)
    statsFULL[:, 0] = 1.0 / C
    KSfull = np.zeros((C, C), np.float32)
    KSfull[:, :NH] = KSblk
    ONESrows = np.ones((C, C), np.float32)                      # rank-1 lhsT rows
    NEGG1rows = np.tile(-g1[None, :].astype(np.float32), (C, 1))

    W1T = np.concatenate([W1[c * 128:(c + 1) * 128, :].T
                          for c in range(4)], axis=1).astype(np.float32)  # (128,512)
    B1c = np.stack([b1[c * 128:(c + 1) * 128] for c in range(4)],
                   axis=1).astype(np.float32)                             # (128,4)
    W2T = np.concatenate([W2[:, c * 128:(c + 1) * 128].T
                          for c in range(4)], axis=1).astype(np.float32)  # (128,512)
    return dict(
        WqxT=Wq[:, :C].T.astype(np.float32).copy(),
        Wqg=Wq[:, C:].astype(np.float32).copy(),
        bq=bq.astype(np.float32),
        KVblk=KVblk, KSblk=KSblk, SelRep=SelRep, Irep=Irep,
        statsONE=statsONE, statsFULL=statsFULL, KSfull=KSfull,
        ONESrows=ONESrows, NEGG1rows=NEGG1rows,
        W1T=W1T, B1c=B1c, W2T=W2T,
        I128=np.eye(C, dtype=np.float32),
    )


def build_bass(ntiles=NTILES):
    """Build the SPMD Bacc program for one core over ntiles*F tokens."""
    import concourse.bacc as bacc
    import concourse.mybir as mybir
    import concourse.tile as tile

    fp32 = mybir.dt.float32
    ntok = ntiles * F
    nc = bacc.Bacc("TRN2", target_bir_lowering=False, debug=False,
                   num_devices=NCORES)

    xT = nc.dram_tensor("xT", [C, ntok], fp32, kind="ExternalInput")
    outT = nc.dram_tensor("outT", [C, ntok], fp32, kind="ExternalOutput")
    d_consts = {}
    for name, shape in [
            ("WqxT", [C, C]), ("qgT", [T, C]), ("KVblk", [C, C]),
            ("KSblk", [C, NH]), ("SelRep", [C, C]), ("Irep", [T, F]),
            ("statsONE", [C, 1]), ("statsFULL", [C, C]), ("KSfull", [C, C]),
            ("ONESrows", [C, C]), ("NEGG1rows", [C, C]),
            ("W1T", [C, 4 * C]), ("B1c", [C, 4]), ("W2T", [C, 4 * C]),
            ("I128", [C, C])]:
        d_consts[name] = nc.dram_tensor(name, shape, fp32, kind="ExternalInput")

    Exp = mybir.ActivationFunctionType.Exp
    Ln = mybir.ActivationFunctionType.Ln
    Gelu = mybir.ActivationFunctionType.Gelu
    Copy = mybir.ActivationFunctionType.Copy
    MULT = mybir.AluOpType.mult
    SUB = mybir.AluOpType.subtract
    ADD = mybir.AluOpType.add

    with tile.TileContext(nc) as tc:
        import contextlib
        ctx = contextlib.ExitStack()
        with ctx:
            cpool = ctx.enter_context(tc.tile_pool(name="consts", bufs=1))
            xp = ctx.enter_context(tc.tile_pool(name="xp", bufs=6))
            sp = ctx.enter_context(tc.tile_pool(name="sp", bufs=3))
            wp = ctx.enter_context(tc.tile_pool(name="wp", bufs=6))
            ap_ = ctx.enter_context(tc.tile_pool(name="ap", bufs=6))
            hp = ctx.enter_context(tc.tile_pool(name="hp", bufs=2))
            op = ctx.enter_context(tc.tile_pool(name="op", bufs=2))
            rp = ctx.enter_context(tc.tile_pool(name="rp", bufs=2))
            ps = ctx.enter_context(tc.tile_pool(name="ps", bufs=1, space="PSUM"))
            ps2 = ctx.enter_context(tc.tile_pool(name="ps2", bufs=2, space="PSUM"))
            psf = ctx.enter_context(tc.tile_pool(name="psf", bufs=2, space="PSUM"))

            # --- resident constants ---
            cb = {}
            for name, t in d_consts.items():
                shape = list(t.shape)
                ct = cpool.tile(shape, fp32, tag=f"c_{name}")
                nc.sync.dma_start(out=ct[:], in_=t[:, :])
                cb[name] = ct
            epsA = cpool.tile([C, 1], fp32, tag="epsA")
            nc.vector.memset(epsA[:], EPS_ATTN)
            epsL = cpool.tile([C, 1], fp32, tag="epsL")
            nc.vector.memset(epsL[:], EPS_LN)

            ACT_EXPSET = []   # ACT insts in the ln/exp table set, in order
            ACT_GELUS = []    # gelu ACT insts

            ngroups = ntiles // GROUP
            for g in range(ngroups):
                # group-level PSUM stats banks (4 tiles @ 32-bnd slots)
                bank_mu1 = ps2.tile([C, F], fp32, tag="stats")
                bank_msq1 = ps2.tile([C, F], fp32, tag="stats")
                bank_z = ps.tile([C, F], fp32, tag="z")

                tiles = range(g * GROUP, (g + 1) * GROUP)
                x_t, x2_t, qf_t = {}, {}, {}
                for t in tiles:
                    j = t % GROUP
                    sl = slice(t * F, (t + 1) * F)
                    xt = xp.tile([C, F], fp32, tag="x")
                    nc.sync.dma_start(out=xt[:], in_=xT[:, sl])
                    x_t[t] = xt
                    # Qproj + guidance
                    psQ = ps2.tile([C, F], fp32, tag="qa")
                    nc.tensor.matmul(psQ[:], cb["WqxT"][:], xt[:],
                                     start=True, stop=False)
                    nc.tensor.matmul(psQ[:], cb["qgT"][:], cb["Irep"][:],
                                     start=False, stop=True)
                    # stats1
                    if j == 0:
                        nc.tensor.matmul(bank_mu1[:, :], cb["statsFULL"][:], xt[:])
                    else:
                        nc.tensor.matmul(bank_mu1[32 * j:32 * j + 1, :],
                                         cb["statsONE"][:], xt[:],
                                         tile_position=(0, 32 * j))
                    x2 = sp.tile([C, F], fp32, tag="x2")
                    ACT_EXPSET.append(nc.scalar.activation(
                        x2[:], xt[:], mybir.ActivationFunctionType.Square))
                    x2_t[t] = x2
                    if j == 0:
                        nc.tensor.matmul(bank_msq1[:, :], cb["statsFULL"][:], x2[:])
                    else:
                        nc.tensor.matmul(bank_msq1[32 * j:32 * j + 1, :],
                                         cb["statsONE"][:], x2[:],
                                         tile_position=(0, 32 * j))
                    # elu+1 = exp(min(q,0)) + relu(q)
                    m = sp.tile([C, F], fp32, tag="m")
                    nc.vector.tensor_scalar_min(m[:], psQ[:], 0.0)
                    r = sp.tile([C, F], fp32, tag="r")
                    nc.vector.tensor_scalar_max(r[:], psQ[:], 0.0)
                    e = sp.tile([C, F], fp32, tag="e")
                    ACT_EXPSET.append(nc.scalar.activation(e[:], m[:], Exp))
                    qf = sp.tile([C, F], fp32, tag="qf")
                    nc.vector.tensor_tensor(qf[:], e[:], r[:], ADD)
                    qf_t[t] = qf
                    # attention numerator + z
                    psA = ps2.tile([C, F], fp32, tag="qa")
                    nc.tensor.matmul(psA[:], cb["KVblk"][:], qf[:])
                    if j == 0:
                        nc.tensor.matmul(bank_z[:, :], cb["KSfull"][:], qf[:])
                    else:
                        nc.tensor.matmul(bank_z[32 * j:32 * j + NH, :],
                                         cb["KSblk"][:], qf[:],
                                         tile_position=(0, 32 * j))
                    attnS = ap_.tile([C, F], fp32, tag="attnS")
                    nc.vector.tensor_copy(attnS[:], psA[:])
                    x_t[t + 1000] = attnS  # stash (dict reuse)

                # ---- rowmath phase 1 (zinv, rstd1, u1) ----
                lnz = rp.tile([C, F], fp32, tag="lnz")
                ACT_EXPSET.append(nc.scalar.activation(
                    lnz[:], bank_z[:], Ln, bias=epsA[:]))
                zinvR = rp.tile([C, F], fp32, tag="zinvR")
                ACT_EXPSET.append(nc.scalar.activation(
                    zinvR[:], lnz[:], Exp, scale=-1.0))
                muS = rp.tile([C, F], fp32, tag="muS")
                ACT_EXPSET.append(nc.scalar.activation(muS[:], bank_mu1[:], Copy))
                musq = rp.tile([C, F], fp32, tag="musq")
                nc.vector.tensor_tensor(musq[:], muS[:], muS[:], MULT)
                var1 = rp.tile([C, F], fp32, tag="var1")
                nc.vector.tensor_tensor(var1[:], bank_msq1[:], musq[:], SUB)
                lnv1 = rp.tile([C, F], fp32, tag="lnv1")
                ACT_EXPSET.append(nc.scalar.activation(
                    lnv1[:], var1[:], Ln, bias=epsL[:]))
                rstd1R = rp.tile([C, F], fp32, tag="rstd1R")
                ACT_EXPSET.append(nc.scalar.activation(
                    rstd1R[:], lnv1[:], Exp, scale=-0.5))
                u1R = rp.tile([C, F], fp32, tag="u1R")
                nc.vector.tensor_tensor(u1R[:], muS[:], rstd1R[:], MULT)

                # ---- per-tile: zb/A1 bcasts, w, stats2 ----
                bank_mu2 = ps2.tile([C, F], fp32, tag="stats")
                bank_msq2 = ps2.tile([C, F], fp32, tag="stats")
                w_t, w2_t = {}, {}
                for t in tiles:
                    j = t % GROUP
                    attnS = x_t[t + 1000]
                    psZB = psf.tile([C, F], fp32, tag="bcf")
                    nc.tensor.matmul(psZB[:], cb["SelRep"][32 * j:32 * j + NH, :],
                                     zinvR[32 * j:32 * j + NH, :],
                                     tile_position=(32 * j, 0))
                    attn = sp.tile([C, F], fp32, tag="attn")
                    nc.vector.tensor_tensor(attn[:], attnS[:], psZB[:], MULT)
                    psA1 = psf.tile([C, F], fp32, tag="bcf")
                    nc.tensor.matmul(psA1[:], cb["ONESrows"][32 * j:32 * j + 1, :],
                                     rstd1R[32 * j:32 * j + 1, :],
                                     tile_position=(32 * j, 0))
                    t1 = sp.tile([C, F], fp32, tag="t1")
                    nc.vector.tensor_tensor(t1[:], x_t[t][:], psA1[:], MULT)
                    w = wp.tile([C, F], fp32, tag="w")
                    nc.vector.tensor_tensor(w[:], attn[:], t1[:], ADD)
                    w_t[t] = w
                    if j == 0:
                        nc.tensor.matmul(bank_mu2[:, :], cb["statsFULL"][:], w[:])
                    else:
                        nc.tensor.matmul(bank_mu2[32 * j:32 * j + 1, :],
                                         cb["statsONE"][:], w[:],
                                         tile_position=(0, 32 * j))
                    w2 = sp.tile([C, F], fp32, tag="w2")
                    ACT_EXPSET.append(nc.scalar.activation(
                        w2[:], w[:], mybir.ActivationFunctionType.Square))
                    w2_t[t] = w2
                    if j == 0:
                        nc.tensor.matmul(bank_msq2[:, :], cb["statsFULL"][:], w2[:])
                    else:
                        nc.tensor.matmul(bank_msq2[32 * j:32 * j + 1, :],
                                         cb["statsONE"][:], w2[:],
                                         tile_position=(0, 32 * j))

                # ---- rowmath phase 2 (mu2w copy, rstd2) ----
                mu2wS = rp.tile([C, F], fp32, tag="mu2wS")
                ACT_EXPSET.append(nc.scalar.activation(mu2wS[:], bank_mu2[:], Copy))
                musq2 = rp.tile([C, F], fp32, tag="musq")
                nc.vector.tensor_tensor(musq2[:], mu2wS[:], mu2wS[:], MULT)
                var2 = rp.tile([C, F], fp32, tag="var1")
                nc.vector.tensor_tensor(var2[:], bank_msq2[:], musq2[:], SUB)
                lnv2 = rp.tile([C, F], fp32, tag="lnv1")
                ACT_EXPSET.append(nc.scalar.activation(
                    lnv2[:], var2[:], Ln, bias=epsL[:]))
                rstd2R = rp.tile([C, F], fp32, tag="rstd2R")
                ACT_EXPSET.append(nc.scalar.activation(
                    rstd2R[:], lnv2[:], Exp, scale=-0.5))

                # ---- per-tile: LN2 apply, FFN, residuals, store ----
                for t in tiles:
                    j = t % GROUP
                    w = w_t[t]
                    psM2 = psf.tile([C, F], fp32, tag="bcf")
                    nc.tensor.matmul(psM2[:], cb["ONESrows"][32 * j:32 * j + 1, :],
                                     mu2wS[32 * j:32 * j + 1, :],
                                     tile_position=(32 * j, 0))
                    ln2p = sp.tile([C, F], fp32, tag="ln2p")
                    nc.vector.tensor_tensor(ln2p[:], w[:], psM2[:], SUB)
                    psA2 = psf.tile([C, F], fp32, tag="bcf")
                    nc.tensor.matmul(psA2[:], cb["ONESrows"][32 * j:32 * j + 1, :],
                                     rstd2R[32 * j:32 * j + 1, :],
                                     tile_position=(32 * j, 0))
                    ln2 = sp.tile([C, F], fp32, tag="ln2")
                    nc.vector.tensor_tensor(ln2[:], ln2p[:], psA2[:], MULT)

                    psOut = ps.tile([C, F], fp32, tag="out")
                    for c in range(4):
                        psF1 = psf.tile([C, F], fp32, tag="bcf")
                        nc.tensor.matmul(psF1[:], cb["W1T"][:, 128 * c:128 * (c + 1)],
                                         ln2[:])
                        h = hp.tile([C, F], fp32, tag="h")
                        ACT_GELUS.append(nc.scalar.activation(
                            h[:], psF1[:], Gelu, bias=cb["B1c"][:, c:c + 1]))
                        nc.tensor.matmul(psOut[:], cb["W2T"][:, 128 * c:128 * (c + 1)],
                                         h[:], start=(c == 0), stop=False,
                                         skip_group_check=True)
                    nc.tensor.matmul(psOut[:], cb["I128"][:], w[:],
                                     start=False, stop=False, skip_group_check=True)
                    nc.tensor.matmul(psOut[:], cb["NEGG1rows"][32 * j:32 * j + 1, :],
                                     u1R[32 * j:32 * j + 1, :],
                                     tile_position=(32 * j, 0),
                                     start=False, stop=True, skip_group_check=True)
                    outS = op.tile([C, F], fp32, tag="outS")
                    nc.vector.tensor_copy(outS[:], psOut[:])
                    nc.sync.dma_start(out=outT[:, t * F:(t + 1) * F], in_=outS[:])

    nc.compile()
    return nc


def _shard_inputs(inputs, consts, ntiles=NTILES):
    """Build per-core in_maps (list of dicts)."""
    x = np.asarray(inputs["x"], np.float32)
    guidance = np.asarray(inputs["guidance"], np.float32)
    ntok = ntiles * F
    in_maps = []
    const_arrs = {k: consts[k] for k in
                  ("WqxT", "KVblk", "KSblk", "SelRep", "Irep", "statsONE",
                   "statsFULL", "KSfull", "ONESrows", "NEGG1rows", "W1T", "B1c",
                   "W2T", "I128")}
    for core in range(NCORES):
        b = core // 2
        h0 = 12 * (core % 2)
        xs = x[b, :, :, h0:h0 + 12, :]                 # (T,C,12,24)
        xc = np.ascontiguousarray(
            xs.transpose(1, 2, 3, 0).reshape(C, NT_CORE))[:, :ntok]
        qg = (guidance[b].astype(np.float64) @ consts["Wqg"].astype(np.float64).T
              + consts["bq"].astype(np.float64)).astype(np.float32)   # (T,C)
        m = {"xT": np.ascontiguousarray(xc), "qgT": qg}
        m.update(const_arrs)
        in_maps.append(m)
    return in_maps


def _unshard(results):
    out = np.empty((B, T, C, Hs, Ws), np.float32)
    for core in range(NCORES):
        b = core // 2
        h0 = 12 * (core % 2)
        o = results[core]["outT"]                       # (C, NT_CORE)
        o4 = o.reshape(C, 12, 24, T).transpose(3, 0, 1, 2)
        out[b, :, :, h0:h0 + 12, :] = o4
    return out


def _numpy_fallback(inputs):
    """Plain-numpy reference path (used only for nontrivial ln g/b)."""
    from scipy.special import erf
    x = np.asarray(inputs["x"], np.float64)
    guidance = np.asarray(inputs["guidance"], np.float64)
    i64 = {k: np.asarray(v, np.float64) for k, v in inputs.items()}
    b_, t_, c_, h_, w_ = x.shape
    n = b_ * h_ * w_
    xb = x.transpose(0, 3, 4, 1, 2).reshape(n, t_, c_)
    g = np.broadcast_to(guidance[:, None, None, :, :],
                        (b_, h_, w_, t_, guidance.shape[-1])).reshape(n, t_, -1)
    q = np.concatenate([xb, g], -1) @ i64["Wq"].T + i64["bq"]
    proto = i64["protos"][0]
    k = proto @ i64["Wk"].T + i64["bk"]
    v = proto @ i64["Wv"].T + i64["bv"]
    elu1 = lambda z: np.where(z > 0, z, np.expm1(z)) + 1.0
    qf = elu1(q.reshape(n, t_, NH, HD))
    kf = elu1(k.reshape(P, NH, HD))
    vv = v.reshape(P, NH, HD) / P
    KV = np.einsum('phd,phv->hdv', kf, vv)
    ksum = kf.sum(0)
    Z = 1.0 / (np.einsum('nlhd,hd->nlh', qf, ksum) + EPS_ATTN)
    out = np.einsum('nlhd,hdv->nlhv', qf, KV) * Z[..., None] * P
    out = out.reshape(n, t_, c_)
    ln = lambda z, gg, bb: ((z - z.mean(-1, keepdims=True))
                            / np.sqrt(z.var(-1, keepdims=True) + EPS_LN) * gg + bb)
    out = out + ln(xb, i64["ln1_g"], i64["ln1_b"])
    hdn = ln(out, i64["ln2_g"], i64["ln2_b"]) @ i64["W1"].T + i64["b1"]
    hdn = 0.5 * hdn * (1.0 + erf(hdn / np.sqrt(2.0)))
    out = out + hdn @ i64["W2"].T + i64["b2"]
    out = out.reshape(b_, h_, w_, t_, c_).transpose(0, 3, 4, 1, 2)
    return out.astype(np.float32)


def kernel(**inputs):
    g1 = np.asarray(inputs["ln1_g"]); b1 = np.asarray(inputs["ln1_b"])
    g2 = np.asarray(inputs["ln2_g"]); b2l = np.asarray(inputs["ln2_b"])
    if not (np.allclose(g1, 1) and np.allclose(g2, 1)
            and np.allclose(b1, 0) and np.allclose(b2l, 0)
            and np.allclose(np.asarray(inputs["b2"]), 0)):
        return _numpy_fallback(inputs)

    from concourse.bass_utils import run_bass_kernel_spmd
    consts = build_consts(inputs)
    key = NTILES
    if key not in _COMPILED:
        _COMPILED[key] = build_bass(NTILES)
    nc = _COMPILED[key]
    in_maps = _shard_inputs(inputs, consts)
    res = run_bass_kernel_spmd(nc, in_maps, list(range(NCORES)))
    return _unshard(res.results)
